# revision 17
# baseline (speedup 1.0000x reference)
"""Trainium2 Bass kernel for nn_AccuracyMetricLoss.

Computes mean over 200000 days of per-day scores:
    denom = max(t, 0.2*cap);  rel_sq = ((t-p)/denom)^2
    score_d = (1 - sqrt(mean_96(rel_sq))) * 100;  out = mean_d(score_d)

Sharding: day axis split evenly across 8 NeuronCores (25000 days/core).

The host repacks the two inputs into one per-core buffer where each DMA
chunk holds interleaved rows [t_row | p_row], so every t/p chunk pair is
a single large DRAM-sequential DMA (minimizes per-DMA ring boundaries).
All chunks stay resident in SBUF (loads never gated on compute) and all
DMAs ride one HWDGE ring (SP). Per chunk:
    ACT:  q = t^-1/2  (Abs_reciprocal_sqrt)
    DVE:  d = t - p                            (in-place into the p half)
    DVE:  custom fused op  s = cumsum(d^2 * min(q, thresh^-1/2)^4)
    DVE:  copy strided per-day prefix samples s[:, 95::96] into acc
    one final DMA of acc -> DRAM
Host: difference the prefix samples -> per-day sums, sqrt/score/mean in f64.
"""
import os
import sys

sys.path.insert(0, "/opt/trn_rl_repo")

import numpy as np

import concourse.bacc as bacc
import concourse.mybir as mybir
from concourse.bass_utils import run_bass_kernel_spmd
from concourse.tile import TileContext

from concourse.dve_ops import DveOp, OPS, CUSTOM_DVE_SPECS, _SUB_OPCODE_FOR_NAME
from concourse.dve_spec import Spec, Src0, Src1, C0, AluOp, sq, minn, scan, lower
from concourse.dve_uop import DveOpSpec

# ---------------- problem constants (hardcoded) ---------------- #
CAP = (300 + 400 + 900) / 300 / 1000 * 300400.0  # 1602.1333...
THRESH = np.float32(0.2) * np.float32(CAP)
CQ = float(np.float64(THRESH) ** -0.5)  # clamp for q = t^-1/2  (q^4 = 1/t^2)
T = 96
N_DAYS = 200000
N_CORES = 8
DAYS_PER_CORE = N_DAYS // N_CORES  # 25000
P = 128
# (rows, days_per_row) per chunk; one DMA and one compute slice per chunk
CHUNKS = [
    (128, 25),
    (128, 50),
    (128, 55),
    (128, 50),
    (128, 15),
    (8, 5),
]
assert sum(r * c for r, c in CHUNKS) == DAYS_PER_CORE
ACC_COLS = sum(c for _, c in CHUNKS)  # 200
MAX_SLICE_FD = max(c for _, c in CHUNKS) * T  # 5280


def _register_clamp_sq_scan():
    # out = cumsum(in0^2 * min(in1, s0)^4): in0 = t-p, in1 = t^-1/2,
    # s0 = thresh^-1/2, so min(in1,s0)^4 = 1/max(t,thresh)^2
    name = "CLAMP4_SQ_SCAN_ANT"
    for op in OPS:
        if op.name == name:
            return op

    qc = minn(Src1, C0)
    body = scan(AluOp.ADD, sq(Src0) * sq(sq(qc)))

    def _ref(in0, in1, s0, s1, imm2):
        x = np.asarray(in0, np.float32)
        r = np.asarray(in1, np.float32).reshape(x.shape[0], -1)
        c = s0 if isinstance(s0, float) else np.asarray(s0, np.float32).reshape(-1, 1)
        b = (x.reshape(x.shape[0], -1) ** 2) * np.minimum(r, c) ** 4
        out = np.cumsum(b.astype(np.float32), axis=-1, dtype=np.float32)
        return out.reshape(in0.shape)

    spec = Spec(body=body, reference=_ref)
    row = 1 + len(OPS)
    assert row < 0x20
    _SUB_OPCODE_FOR_NAME[name] = row
    shas = {}
    for ver in ("v3", "v4"):
        u = lower(spec, ver=ver)
        shas[ver] = DveOpSpec(name=name, opcode=row, uops=u, rd1_en=True).sha(ver)
    op = DveOp(name, spec, subdim=False, uops_sha=shas)
    OPS.append(op)
    CUSTOM_DVE_SPECS[name] = spec
    return op


_nc_cache = {}


def _build_nc():
    if "nc" in _nc_cache:
        return _nc_cache["nc"]
    clamp_sq_scan = _register_clamp_sq_scan()

    nc = bacc.Bacc("TRN2")
    n_elem = DAYS_PER_CORE * T
    tp_in = nc.dram_tensor(
        "tp_in", [2 * n_elem], mybir.dt.float32, kind="ExternalInput"
    )
    out = nc.dram_tensor("out", [P, ACC_COLS], mybir.dt.float32, kind="ExternalOutput")

    with TileContext(nc) as tc:
        with (
            tc.tile_pool(name="tp", bufs=1) as tp,
            tc.tile_pool(name="lp", bufs=2) as lp,
            tc.tile_pool(name="accp", bufs=1) as accp,
        ):
            acc = accp.tile([P, ACC_COLS], mybir.dt.float32)
            # all chunks resident; one merged [t_row|p_row] DMA per chunk,
            # all on the SP ring in ladder order
            tiles = []
            base = 0
            for ci, (rows, cdays) in enumerate(CHUNKS):
                fd = cdays * T
                tile = tp.tile([P, 2 * fd], mybir.dt.float32, tag=f"c{ci}")
                n = rows * 2 * fd
                v = tp_in[base : base + n].rearrange("(p f) -> p f", p=rows)
                nc.sync.dma_start(out=tile[:rows, :], in_=v)
                tiles.append(tile)
                base += n
            acc_col = 0
            for ci, (rows, cdays) in enumerate(CHUNKS):
                fd = cdays * T
                ts = tiles[ci][:rows, 0:fd]
                ps = tiles[ci][:rows, fd : 2 * fd]
                lt = lp.tile([P, MAX_SLICE_FD], mybir.dt.float32, tag="lt")
                lts = lt[:rows, :fd]
                # q = t^-1/2
                nc.scalar.activation(
                    lts, ts, mybir.ActivationFunctionType.Abs_reciprocal_sqrt
                )
                # d = t - p   (in place into the p half)
                nc.vector.tensor_tensor(ps, ts, ps, mybir.AluOpType.subtract)
                # s = cumsum(d^2 * min(q, CQ)^4)  (into the t half: dead now)
                nc.vector._custom_dve(clamp_sq_scan, out=ts, in0=ps, in1=lts, s0=CQ)
                # collect per-day prefix samples into acc
                samples = ts.rearrange("p (c n) -> p c n", n=T)[:, :, 95]
                nc.vector.tensor_copy(acc[:rows, acc_col : acc_col + cdays], samples)
                acc_col += cdays
            nc.sync.dma_start(out=out[:], in_=acc[:])
    nc.finalize()
    _nc_cache["nc"] = nc
    return nc


def _repack(true_s: np.ndarray, pred_s: np.ndarray) -> np.ndarray:
    """Interleave per-chunk rows as [t_row | p_row] into one flat buffer."""
    n_elem = DAYS_PER_CORE * T
    buf = np.empty(2 * n_elem, dtype=np.float32)
    src = 0
    dst = 0
    for rows, cdays in CHUNKS:
        fd = cdays * T
        n = rows * fd
        pair = buf[dst : dst + 2 * n].reshape(rows, 2 * fd)
        pair[:, :fd] = true_s[src : src + n].reshape(rows, fd)
        pair[:, fd:] = pred_s[src : src + n].reshape(rows, fd)
        src += n
        dst += 2 * n
    return buf


_last_results = None


def kernel(pred: np.ndarray, true: np.ndarray) -> np.ndarray:
    global _last_results
    nc = _build_nc()

    n_elem = DAYS_PER_CORE * T
    pred = np.ascontiguousarray(pred, dtype=np.float32)
    true = np.ascontiguousarray(true, dtype=np.float32)
    in_maps = [
        {
            "tp_in": _repack(
                true[k * n_elem : (k + 1) * n_elem],
                pred[k * n_elem : (k + 1) * n_elem],
            )
        }
        for k in range(N_CORES)
    ]

    trace = False
    if os.environ.get("BASS_TRACE"):
        try:  # tracing needs the axon NTFF hook; never crash without it
            import antenv.axon_hooks  # noqa: F401

            trace = True
        except ImportError:
            pass
    res = run_bass_kernel_spmd(nc, in_maps, list(range(N_CORES)), trace=trace)
    _last_results = res

    # host-side tail: prefix samples -> day sums -> scores -> mean
    total = 0.0
    for k in range(N_CORES):
        A = res.results[k]["out"].astype(np.float64)  # [128, ACC_COLS]
        acc_col = 0
        for rows, cdays in CHUNKS:
            S = A[:rows, acc_col : acc_col + cdays]
            u = S.copy()
            u[:, 1:] -= S[:, :-1]  # per-day sums of rel_sq
            np.maximum(u, 0.0, out=u)  # guard sqrt against diff rounding
            scores = (1.0 - np.sqrt(u / T)) * 100.0
            total += scores.sum()
            acc_col += cdays
    return np.float32(total / N_DAYS)


# revision 18
# speedup vs baseline: 1.0957x; 1.0957x over previous
"""Trainium2 Bass kernel for nn_AccuracyMetricLoss.

Computes mean over 200000 days of per-day scores:
    denom = max(t, 0.2*cap);  rel_sq = ((t-p)/denom)^2
    score_d = (1 - sqrt(mean_96(rel_sq))) * 100;  out = mean_d(score_d)

Sharding: day axis split evenly across 8 NeuronCores (25000 days/core).

The host repacks the two inputs into one per-core buffer where each DMA
chunk holds interleaved rows [t_row | p_row], so every t/p chunk pair is
a single large DRAM-sequential DMA (minimizes per-DMA ring boundaries).
All chunks stay resident in SBUF (loads never gated on compute) and all
DMAs ride one HWDGE ring (SP). Per chunk:
    ACT:  q = t^-1/2  (Abs_reciprocal_sqrt)
    DVE:  d = t - p                            (in-place into the p half)
    DVE:  custom fused op  s = cumsum(d^2 * min(q, thresh^-1/2)^4)
    DVE:  copy strided per-day prefix samples s[:, 95::96] into acc
    one final DMA of acc -> DRAM
Host: difference the prefix samples -> per-day sums, sqrt/score/mean in f64.
"""
import os
import sys

sys.path.insert(0, "/opt/trn_rl_repo")

import numpy as np

import concourse.bacc as bacc
import concourse.mybir as mybir
from concourse.bass_utils import run_bass_kernel_spmd
from concourse.tile import TileContext

from concourse.dve_ops import DveOp, OPS, CUSTOM_DVE_SPECS, _SUB_OPCODE_FOR_NAME
from concourse.dve_spec import Spec, Src0, Src1, C0, AluOp, sq, minn, scan, lower
from concourse.dve_uop import DveOpSpec

# ---------------- problem constants (hardcoded) ---------------- #
CAP = (300 + 400 + 900) / 300 / 1000 * 300400.0  # 1602.1333...
THRESH = np.float32(0.2) * np.float32(CAP)
CQ = float(np.float64(THRESH) ** -0.5)  # clamp for q = t^-1/2  (q^4 = 1/t^2)
T = 96
N_DAYS = 200000
N_CORES = 8
DAYS_PER_CORE = N_DAYS // N_CORES  # 25000
P = 128
# (rows, days_per_row) per chunk; one DMA and one compute slice per chunk
CHUNKS = [
    (128, 25),
    (128, 50),
    (128, 55),
    (128, 50),
    (128, 15),
    (8, 5),
]
assert sum(r * c for r, c in CHUNKS) == DAYS_PER_CORE
ACC_COLS = sum(c for _, c in CHUNKS)  # 200
MAX_SLICE_FD = max(c for _, c in CHUNKS) * T  # 5280
PAD = 8  # floats between the t and p halves (breaks SBUF bank parity)


def _register_clamp_sq_scan():
    # out = cumsum(in0^2 * min(in1, s0)^4): in0 = t-p, in1 = t^-1/2,
    # s0 = thresh^-1/2, so min(in1,s0)^4 = 1/max(t,thresh)^2
    name = "CLAMP4_SQ_SCAN_ANT"
    for op in OPS:
        if op.name == name:
            return op

    qc = minn(Src1, C0)
    body = scan(AluOp.ADD, sq(Src0) * sq(sq(qc)))

    def _ref(in0, in1, s0, s1, imm2):
        x = np.asarray(in0, np.float32)
        r = np.asarray(in1, np.float32).reshape(x.shape[0], -1)
        c = s0 if isinstance(s0, float) else np.asarray(s0, np.float32).reshape(-1, 1)
        b = (x.reshape(x.shape[0], -1) ** 2) * np.minimum(r, c) ** 4
        out = np.cumsum(b.astype(np.float32), axis=-1, dtype=np.float32)
        return out.reshape(in0.shape)

    spec = Spec(body=body, reference=_ref)
    row = 1 + len(OPS)
    assert row < 0x20
    _SUB_OPCODE_FOR_NAME[name] = row
    shas = {}
    for ver in ("v3", "v4"):
        u = lower(spec, ver=ver)
        shas[ver] = DveOpSpec(name=name, opcode=row, uops=u, rd1_en=True).sha(ver)
    op = DveOp(name, spec, subdim=False, uops_sha=shas)
    OPS.append(op)
    CUSTOM_DVE_SPECS[name] = spec
    return op


_nc_cache = {}


def _build_nc():
    if "nc" in _nc_cache:
        return _nc_cache["nc"]
    clamp_sq_scan = _register_clamp_sq_scan()

    nc = bacc.Bacc("TRN2")
    n_elem = DAYS_PER_CORE * T
    pad_total = sum(rows * PAD for rows, _ in CHUNKS)
    tp_in = nc.dram_tensor(
        "tp_in", [2 * n_elem + pad_total], mybir.dt.float32, kind="ExternalInput"
    )
    out = nc.dram_tensor("out", [P, ACC_COLS], mybir.dt.float32, kind="ExternalOutput")

    with TileContext(nc) as tc:
        with (
            tc.tile_pool(name="tp", bufs=1) as tp,
            tc.tile_pool(name="lp", bufs=2) as lp,
            tc.tile_pool(name="accp", bufs=1) as accp,
        ):
            acc = accp.tile([P, ACC_COLS], mybir.dt.float32)
            # all chunks resident; one merged [t_row|p_row] DMA per chunk,
            # all on the SP ring in ladder order
            tiles = []
            base = 0
            for ci, (rows, cdays) in enumerate(CHUNKS):
                fd = cdays * T
                w = 2 * fd + PAD
                tile = tp.tile([P, w], mybir.dt.float32, tag=f"c{ci}")
                n = rows * w
                v = tp_in[base : base + n].rearrange("(p f) -> p f", p=rows)
                nc.sync.dma_start(out=tile[:rows, :], in_=v)
                tiles.append(tile)
                base += n
            acc_col = 0
            for ci, (rows, cdays) in enumerate(CHUNKS):
                fd = cdays * T
                ts = tiles[ci][:rows, 0:fd]
                ps = tiles[ci][:rows, fd + PAD : 2 * fd + PAD]
                lt = lp.tile([P, MAX_SLICE_FD], mybir.dt.float32, tag="lt")
                lts = lt[:rows, :fd]
                # q = t^-1/2
                nc.scalar.activation(
                    lts, ts, mybir.ActivationFunctionType.Abs_reciprocal_sqrt
                )
                # d = t - p   (in place into the p half)
                nc.vector.tensor_tensor(ps, ts, ps, mybir.AluOpType.subtract)
                # s = cumsum(d^2 * min(q, CQ)^4)  (into the t half: dead now)
                nc.vector._custom_dve(clamp_sq_scan, out=ts, in0=ps, in1=lts, s0=CQ)
                # collect per-day prefix samples into acc
                samples = ts.rearrange("p (c n) -> p c n", n=T)[:, :, 95]
                nc.vector.tensor_copy(acc[:rows, acc_col : acc_col + cdays], samples)
                acc_col += cdays
            nc.sync.dma_start(out=out[:], in_=acc[:])
    nc.finalize()
    _nc_cache["nc"] = nc
    return nc


def _repack(true_s: np.ndarray, pred_s: np.ndarray) -> np.ndarray:
    """Interleave per-chunk rows as [t_row | pad | p_row] into one buffer."""
    n_elem = DAYS_PER_CORE * T
    pad_total = sum(rows * PAD for rows, _ in CHUNKS)
    buf = np.zeros(2 * n_elem + pad_total, dtype=np.float32)
    src = 0
    dst = 0
    for rows, cdays in CHUNKS:
        fd = cdays * T
        w = 2 * fd + PAD
        n = rows * fd
        pair = buf[dst : dst + rows * w].reshape(rows, w)
        pair[:, :fd] = true_s[src : src + n].reshape(rows, fd)
        pair[:, fd + PAD :] = pred_s[src : src + n].reshape(rows, fd)
        src += n
        dst += rows * w
    return buf


_last_results = None


def kernel(pred: np.ndarray, true: np.ndarray) -> np.ndarray:
    global _last_results
    nc = _build_nc()

    n_elem = DAYS_PER_CORE * T
    pred = np.ascontiguousarray(pred, dtype=np.float32)
    true = np.ascontiguousarray(true, dtype=np.float32)
    in_maps = [
        {
            "tp_in": _repack(
                true[k * n_elem : (k + 1) * n_elem],
                pred[k * n_elem : (k + 1) * n_elem],
            )
        }
        for k in range(N_CORES)
    ]

    trace = False
    if os.environ.get("BASS_TRACE"):
        try:  # tracing needs the axon NTFF hook; never crash without it
            import antenv.axon_hooks  # noqa: F401

            trace = True
        except ImportError:
            pass
    res = run_bass_kernel_spmd(nc, in_maps, list(range(N_CORES)), trace=trace)
    _last_results = res

    # host-side tail: prefix samples -> day sums -> scores -> mean
    total = 0.0
    for k in range(N_CORES):
        A = res.results[k]["out"].astype(np.float64)  # [128, ACC_COLS]
        acc_col = 0
        for rows, cdays in CHUNKS:
            S = A[:rows, acc_col : acc_col + cdays]
            u = S.copy()
            u[:, 1:] -= S[:, :-1]  # per-day sums of rel_sq
            np.maximum(u, 0.0, out=u)  # guard sqrt against diff rounding
            scores = (1.0 - np.sqrt(u / T)) * 100.0
            total += scores.sum()
            acc_col += cdays
    return np.float32(total / N_DAYS)
